# revision 6
# baseline (speedup 1.0000x reference)
"""Trainium2 Bass kernel for AllGNN message passing.

Computes, for full inputs:
    h   = x @ W_in + b_in
    deg = adj.sum(axis=1, keepdims=True)
    agg = (adj @ h) / (deg + 1)
    out = agg @ W_cls + b_cls

Key algebra: row scaling commutes with the right matmul, so
    out = (adj @ G)[:, :C] / (deg+1) + b_cls
with G = [x @ (W_in @ W_cls) + b_in @ W_cls | ones]  (C = n_cls columns + 1
ones column whose product recovers deg).

Sharding: row-shard adj/x over 8 cores. Each core computes g for its own
rows, AllGathers g (small), then streams its adj row-block once from HBM
(cast fp32->bf16 in the DMA; adj is 0/1 so bf16 is exact), transposes
128x128 blocks on the PE (matmul against identity), and accumulates
out.T = G.T @ adj.T in PSUM with G as the stationary operand.
"""

import numpy as np

import concourse.bass as bass
from concourse import bacc
import concourse.mybir as mybir
import concourse.tile as tile
from concourse.bass_utils import run_bass_kernel_spmd

try:
    import ml_dtypes
except ImportError:  # pragma: no cover
    ml_dtypes = None

N_CORES = 8
N_NODES = 12000
IN_CH = 256
HID = 64
N_CLS = 40

JW = 128  # j (contraction) tile width
IW = 128  # i (output-row) tile width


def _ceil_div(a, b):
    return -(-a // b)


def build_gnn(
    n_nodes=N_NODES,
    n_cores=N_CORES,
    in_ch=IN_CH,
    hid=HID,
    n_cls=N_CLS,
    stage_jtiles=24,
    group_its=4,
    strip_bufs=None,
    act_copy_every=3,
):
    f32 = mybir.dt.float32
    bf16 = mybir.dt.bfloat16
    mult = mybir.AluOpType.mult
    add = mybir.AluOpType.add

    assert n_nodes % n_cores == 0
    rows = n_nodes // n_cores
    assert in_ch % 128 == 0
    n_kt = in_ch // 128
    assert hid <= 128 and n_cls + 1 <= 128
    gc = n_cls + 1  # G columns: [g | ones]

    n_jt = _ceil_div(n_nodes, JW)
    n_it = _ceil_div(rows, IW)
    stage_cols = stage_jtiles * JW
    n_chunks = _ceil_div(n_nodes, stage_cols)

    nc = bacc.Bacc(num_devices=n_cores)

    adj_h = nc.dram_tensor("adj_blk", [rows, n_nodes], f32, kind="ExternalInput")
    x_h = nc.dram_tensor("x_blk", [rows, in_ch], f32, kind="ExternalInput")
    win_h = nc.dram_tensor("W_in", [in_ch, hid], f32, kind="ExternalInput")
    bin_h = nc.dram_tensor("b_in", [hid], f32, kind="ExternalInput")
    wcls_h = nc.dram_tensor("W_cls", [hid, n_cls], f32, kind="ExternalInput")
    bcls_h = nc.dram_tensor("b_cls", [n_cls], f32, kind="ExternalInput")
    out_h = nc.dram_tensor("out_blk", [rows, n_cls], f32, kind="ExternalOutput")

    g_local_h = nc.dram_tensor("g_local", [rows, gc], bf16)
    g_full_h = nc.dram_tensor("g_full", [n_nodes, gc], bf16, addr_space="Shared")

    id_f_dram = nc.inline_tensor(np.eye(128, dtype=np.float32), name="ident_f32")
    id_b_dram = nc.inline_tensor(
        np.eye(128).astype(ml_dtypes.bfloat16), name="ident_bf16"
    )

    with tile.TileContext(nc) as tc:
        with tc.tile_pool(name="singles", bufs=1) as singles:
            id_f = singles.tile([128, 128], f32, tag="id_f")
            nc.sync.dma_start(out=id_f, in_=id_f_dram[:])
            id_b = singles.tile([128, 128], bf16, tag="id_b")
            nc.sync.dma_start(out=id_b, in_=id_b_dram[:])
            # b_cls broadcast across partitions
            bcls_sb = singles.tile([128, n_cls], f32, tag="bcls")
            bc = bcls_h[:]
            nc.gpsimd.dma_start(
                out=bcls_sb,
                in_=bass.AP(tensor=bc.tensor, offset=bc.offset, ap=[[0, 128]] + bc.ap),
            )
            G_sb = singles.tile([128, n_jt, gc], bf16, tag="G")

            # ---- Phase A: g = x @ (W_in @ W_cls) + b_in @ W_cls, [g|1] ----
            with (
                tc.tile_pool(name="ph_a", bufs=2) as pa,
                tc.tile_pool(name="ph_a_ps", bufs=4, space="PSUM") as pap,
            ):
                win_sb = pa.tile([128, n_kt, hid], f32, tag="win")
                nc.sync.dma_start(
                    out=win_sb, in_=win_h[:].rearrange("(t p) h -> p t h", p=128)
                )
                wcls_sb = pa.tile([hid, n_cls], f32, tag="wcls")
                nc.sync.dma_start(out=wcls_sb, in_=wcls_h[:])
                bin_sb = pa.tile([hid, 1], f32, tag="bin")
                bi = bin_h[:]
                nc.sync.dma_start(
                    out=bin_sb,
                    in_=bass.AP(tensor=bi.tensor, offset=bi.offset, ap=bi.ap + [[0, 1]]),
                )
                ones_sb = pa.tile([1, 128], f32, tag="ones")
                nc.vector.memset(ones_sb, 1.0)

                # W_in.T tiles via PE transpose (fp32)
                winT_sb = pa.tile([hid, n_kt, 128], f32, tag="winT")
                for t in range(n_kt):
                    ps = pap.tile([hid, 128], f32, tag="ps_a")
                    nc.tensor.matmul(
                        ps, lhsT=win_sb[:, t, :], rhs=id_f, start=True, stop=True
                    )
                    nc.vector.tensor_copy(winT_sb[:, t, :], ps)
                # W2 = W_in @ W_cls  [in_ch, n_cls]
                w2_sb = pa.tile([128, n_kt, n_cls], f32, tag="w2")
                for t in range(n_kt):
                    ps = pap.tile([128, n_cls], f32, tag="ps_a")
                    nc.tensor.matmul(
                        ps, lhsT=winT_sb[:, t, :], rhs=wcls_sb, start=True, stop=True
                    )
                    nc.vector.tensor_copy(w2_sb[:, t, :], ps)
                # b2 = b_in @ W_cls, broadcast to [128, n_cls]
                ps_b2 = pap.tile([1, n_cls], f32, tag="ps_a")
                nc.tensor.matmul(ps_b2, lhsT=bin_sb, rhs=wcls_sb, start=True, stop=True)
                b2row = pa.tile([1, n_cls], f32, tag="b2row")
                nc.vector.tensor_copy(b2row, ps_b2)
                ps_b2b = pap.tile([128, n_cls], f32, tag="ps_a")
                nc.tensor.matmul(ps_b2b, lhsT=ones_sb, rhs=b2row, start=True, stop=True)
                b2b_sb = pa.tile([128, n_cls], f32, tag="b2b")
                nc.vector.tensor_copy(b2b_sb, ps_b2b)

                for it in range(n_it):
                    i0 = it * IW
                    p = min(IW, rows - i0)
                    x_t = pa.tile([128, in_ch], f32, tag="x_t")
                    nc.sync.dma_start(out=x_t[:p], in_=x_h[i0 : i0 + p, :])
                    xT = pa.tile([128, n_kt, 128], f32, tag="xT")
                    for t in range(n_kt):
                        ps = pap.tile([128, 128], f32, tag="ps_a")
                        nc.tensor.matmul(
                            ps[:, :p],
                            lhsT=x_t[:p, t * 128 : (t + 1) * 128],
                            rhs=id_f[:p, :p],
                            start=True,
                            stop=True,
                        )
                        nc.vector.tensor_copy(xT[:, t, :p], ps[:, :p])
                    ps_g = pap.tile([128, n_cls], f32, tag="ps_a")
                    for t in range(n_kt):
                        nc.tensor.matmul(
                            ps_g[:p],
                            lhsT=xT[:, t, :p],
                            rhs=w2_sb[:, t, :],
                            start=(t == 0),
                            stop=(t == n_kt - 1),
                        )
                    g_t = pa.tile([128, gc], bf16, tag="g_t")
                    nc.vector.tensor_add(g_t[:p, 0:n_cls], ps_g[:p], b2b_sb[:p])
                    nc.vector.memset(g_t[:p, n_cls:gc], 1.0)
                    nc.sync.dma_start(out=g_local_h[i0 : i0 + p, :], in_=g_t[:p])

            nc.gpsimd.collective_compute(
                "AllGather",
                mybir.AluOpType.bypass,
                replica_groups=[list(range(n_cores))],
                ins=[g_local_h[:]],
                outs=[g_full_h[:]],
            )

            full_jt = n_nodes // JW
            jtail = n_nodes - full_jt * JW
            if full_jt:
                nc.sync.dma_start(
                    out=G_sb[:, 0:full_jt, :],
                    in_=g_full_h[0 : full_jt * JW, :].rearrange(
                        "(jt p) c -> p jt c", p=JW
                    ),
                )
            if jtail:
                nc.sync.dma_start(
                    out=G_sb[0:jtail, full_jt, :],
                    in_=g_full_h[full_jt * JW : n_nodes, :],
                )

            # ---- Phase B: stream adj, transpose on PE, accumulate out.T ----
            # i-tiles are processed in groups of `group_its`; each group's
            # transposed strips are [jw, group_w] so the accumulating matmul
            # streams up to 512 columns per instruction.
            n_groups = _ceil_div(n_it, group_its)
            if strip_bufs is None:
                sbufs = n_jt + 8
            else:
                sbufs = strip_bufs
            group_w_max = min(group_its * IW, 512)
            assert group_its * IW <= 512
            with (
                tc.tile_pool(name="nat", bufs=8) as nat_pool,
                tc.tile_pool(name="strip", bufs=sbufs) as strip_pool,
                tc.tile_pool(name="outp", bufs=3) as out_pool,
                tc.tile_pool(name="pt", bufs=4, space="PSUM") as pt_pool,
                tc.tile_pool(name="acc", bufs=2, space="PSUM") as acc_pool,
                tc.tile_pool(name="fin", bufs=2, space="PSUM") as fin_pool,
            ):
                copy_ctr = 0
                for grp in range(n_groups):
                    its = list(range(grp * group_its, min((grp + 1) * group_its, n_it)))
                    widths = [min(IW, rows - it * IW) for it in its]
                    offs = [sum(widths[:k]) for k in range(len(its))]
                    gw = sum(widths)
                    strips = [
                        strip_pool.tile([128, group_w_max], bf16, tag="strip")
                        for _ in range(n_jt)
                    ]
                    for k, it in enumerate(its):
                        i0 = it * IW
                        p = widths[k]
                        go = offs[k]
                        # staged cast loads (fp32 -> bf16 in the SWDGE DMA)
                        nat_tiles = []
                        for c in range(n_chunks):
                            j0 = c * stage_cols
                            cw = min(stage_cols, n_nodes - j0)
                            nt_ = nat_pool.tile([128, stage_cols], bf16, tag="nat")
                            nc.gpsimd.dma_start(
                                out=nt_[:p, :cw], in_=adj_h[i0 : i0 + p, j0 : j0 + cw]
                            )
                            nat_tiles.append(nt_)
                        # PE transposes of 128x128 blocks, copy PSUM -> strips
                        for jt in range(n_jt):
                            jw = min(JW, n_nodes - jt * JW)
                            c = jt // stage_jtiles
                            off = jt * JW - c * stage_cols
                            nt_ = nat_tiles[c]
                            ps = pt_pool.tile([128, 128], f32, tag="pt")
                            nc.tensor.matmul(
                                ps[:jw, :p],
                                lhsT=nt_[:p, off : off + jw],
                                rhs=id_b[:p, :p],
                                start=True,
                                stop=True,
                            )
                            st = strips[jt]
                            if act_copy_every and copy_ctr % act_copy_every == (
                                act_copy_every - 1
                            ):
                                nc.scalar.copy(st[:jw, go : go + p], ps[:jw, :p])
                            else:
                                nc.vector.tensor_copy(st[:jw, go : go + p], ps[:jw, :p])
                            copy_ctr += 1
                    # accumulate out.T[:, group cols] over all j tiles
                    ps_acc = acc_pool.tile([gc, group_w_max], f32, tag="acc")
                    for jt in range(n_jt):
                        jw = min(JW, n_nodes - jt * JW)
                        nc.tensor.matmul(
                            ps_acc[:, :gw],
                            lhsT=G_sb[:jw, jt, :],
                            rhs=strips[jt][:jw, :gw],
                            start=(jt == 0),
                            stop=(jt == n_jt - 1),
                        )
                    # finalize per i-tile: U.T, scale by 1/(deg+1), + b_cls
                    U_sb = out_pool.tile([gc, group_w_max], f32, tag="U")
                    nc.vector.tensor_copy(U_sb[:, :gw], ps_acc[:, :gw])
                    for k, it in enumerate(its):
                        i0 = it * IW
                        p = widths[k]
                        go = offs[k]
                        ps_f = fin_pool.tile([128, gc], f32, tag="fin")
                        nc.tensor.matmul(
                            ps_f[:p, :],
                            lhsT=U_sb[:, go : go + p],
                            rhs=id_f[:gc, :gc],
                            start=True,
                            stop=True,
                        )
                        deg1 = out_pool.tile([128, 1], f32, tag="deg1")
                        nc.vector.tensor_scalar_add(deg1[:p], ps_f[:p, n_cls:gc], 1.0)
                        rcp = out_pool.tile([128, 1], f32, tag="rcp")
                        nc.vector.reciprocal(rcp[:p], deg1[:p])
                        o_sb = out_pool.tile([128, n_cls], f32, tag="o")
                        nc.vector.scalar_tensor_tensor(
                            out=o_sb[:p],
                            in0=ps_f[:p, 0:n_cls],
                            scalar=rcp[:p],
                            in1=bcls_sb[:p],
                            op0=mult,
                            op1=add,
                        )
                        nc.sync.dma_start(out=out_h[i0 : i0 + p, :], in_=o_sb[:p])

    nc.compile()
    return nc


_CACHE = {}


def _get_nc():
    if "nc" not in _CACHE:
        _CACHE["nc"] = build_gnn()
    return _CACHE["nc"]


def kernel(x, adj, W_in, b_in, W_cls, b_cls):
    x = np.asarray(x, dtype=np.float32)
    adj = np.asarray(adj, dtype=np.float32)
    W_in = np.asarray(W_in, dtype=np.float32)
    b_in = np.asarray(b_in, dtype=np.float32)
    W_cls = np.asarray(W_cls, dtype=np.float32)
    b_cls = np.asarray(b_cls, dtype=np.float32)

    nc = _get_nc()
    rows = adj.shape[0] // N_CORES
    in_maps = []
    for c in range(N_CORES):
        sl = slice(c * rows, (c + 1) * rows)
        in_maps.append(
            {
                "adj_blk": np.ascontiguousarray(adj[sl]),
                "x_blk": np.ascontiguousarray(x[sl]),
                "W_in": W_in,
                "b_in": b_in,
                "W_cls": W_cls,
                "b_cls": b_cls,
            }
        )
    res = run_bass_kernel_spmd(nc, in_maps, core_ids=list(range(N_CORES)))
    outs = [res.results[c]["out_blk"] for c in range(N_CORES)]
    return np.concatenate(outs, axis=0).astype(np.float32)


# revision 11
# speedup vs baseline: 1.5551x; 1.5551x over previous
"""Trainium2 Bass kernel for AllGNN message passing.

Computes, for full inputs:
    h   = x @ W_in + b_in
    deg = adj.sum(axis=1, keepdims=True)
    agg = (adj @ h) / (deg + 1)
    out = agg @ W_cls + b_cls

Key algebra: row scaling commutes with the right matmul, so
    out = (adj @ G)[:, :C] / (deg+1) + b_cls
with G = [x @ (W_in @ W_cls) + b_in @ W_cls | ones]  (C = n_cls columns + 1
ones column whose product recovers deg).

Sharding: row-shard adj/x over 8 cores. Each core computes g for its own
rows, AllGathers g (small), then streams its adj row-block once from HBM
(cast fp32->bf16 in the DMA; adj is 0/1 so bf16 is exact), transposes
128x128 blocks on the PE (matmul against identity), and accumulates
out.T = G.T @ adj.T in PSUM with G as the stationary operand.
"""

import numpy as np

import concourse.bass as bass
from concourse import bacc
import concourse.mybir as mybir
import concourse.tile as tile
from concourse.bass_utils import run_bass_kernel_spmd

try:
    import ml_dtypes
except ImportError:  # pragma: no cover
    ml_dtypes = None

N_CORES = 8
N_NODES = 12000
IN_CH = 256
HID = 64
N_CLS = 40

JW = 128  # j (contraction) tile width
IW = 128  # i (output-row) tile width


def _ceil_div(a, b):
    return -(-a // b)


def build_gnn(
    n_nodes=N_NODES,
    n_cores=N_CORES,
    in_ch=IN_CH,
    hid=HID,
    n_cls=N_CLS,
    stage_jtiles=12,
    group_its=4,
    strip_bufs=None,
    act_copy_every=3,
    use_is_transpose=True,
):
    f32 = mybir.dt.float32
    bf16 = mybir.dt.bfloat16
    mult = mybir.AluOpType.mult
    add = mybir.AluOpType.add

    assert n_nodes % n_cores == 0
    rows = n_nodes // n_cores
    assert in_ch % 128 == 0
    n_kt = in_ch // 128
    assert hid <= 128 and n_cls + 1 <= 128
    gc = n_cls + 1  # G columns: [g | ones]

    n_jt = _ceil_div(n_nodes, JW)
    n_it = _ceil_div(rows, IW)
    stage_cols = stage_jtiles * JW
    n_chunks = _ceil_div(n_nodes, stage_cols)

    nc = bacc.Bacc(num_devices=n_cores)

    adj_h = nc.dram_tensor("adj_blk", [rows, n_nodes], f32, kind="ExternalInput")
    x_h = nc.dram_tensor("x_blk", [rows, in_ch], f32, kind="ExternalInput")
    win_h = nc.dram_tensor("W_in", [in_ch, hid], f32, kind="ExternalInput")
    bin_h = nc.dram_tensor("b_in", [hid], f32, kind="ExternalInput")
    wcls_h = nc.dram_tensor("W_cls", [hid, n_cls], f32, kind="ExternalInput")
    bcls_h = nc.dram_tensor("b_cls", [n_cls], f32, kind="ExternalInput")
    out_h = nc.dram_tensor("out_blk", [rows, n_cls], f32, kind="ExternalOutput")

    g_local_h = nc.dram_tensor("g_local", [rows, gc], bf16)
    g_full_h = nc.dram_tensor("g_full", [n_nodes, gc], bf16, addr_space="Shared")

    id_f_dram = nc.inline_tensor(np.eye(128, dtype=np.float32), name="ident_f32")
    id_b_dram = nc.inline_tensor(
        np.eye(128).astype(ml_dtypes.bfloat16), name="ident_bf16"
    )

    with tile.TileContext(nc) as tc:
        with tc.tile_pool(name="singles", bufs=1) as singles:
            id_f = singles.tile([128, 128], f32, tag="id_f")
            nc.sync.dma_start(out=id_f, in_=id_f_dram[:])
            id_b = singles.tile([128, 128], bf16, tag="id_b")
            nc.sync.dma_start(out=id_b, in_=id_b_dram[:])
            # b_cls broadcast across partitions
            bcls_sb = singles.tile([128, n_cls], f32, tag="bcls")
            bc = bcls_h[:]
            nc.gpsimd.dma_start(
                out=bcls_sb,
                in_=bass.AP(tensor=bc.tensor, offset=bc.offset, ap=[[0, 128]] + bc.ap),
            )
            G_sb = singles.tile([128, n_jt, gc], bf16, tag="G")

            # ---- Phase A: g = x @ (W_in @ W_cls) + b_in @ W_cls, [g|1] ----
            with (
                tc.tile_pool(name="ph_a", bufs=2) as pa,
                tc.tile_pool(name="ph_a_ps", bufs=4, space="PSUM") as pap,
            ):
                win_sb = pa.tile([128, n_kt, hid], f32, tag="win")
                nc.sync.dma_start(
                    out=win_sb, in_=win_h[:].rearrange("(t p) h -> p t h", p=128)
                )
                wcls_sb = pa.tile([hid, n_cls], f32, tag="wcls")
                nc.sync.dma_start(out=wcls_sb, in_=wcls_h[:])
                bin_sb = pa.tile([hid, 1], f32, tag="bin")
                bi = bin_h[:]
                nc.sync.dma_start(
                    out=bin_sb,
                    in_=bass.AP(tensor=bi.tensor, offset=bi.offset, ap=bi.ap + [[0, 1]]),
                )
                ones_sb = pa.tile([1, 128], f32, tag="ones")
                nc.vector.memset(ones_sb, 1.0)

                # W_in.T tiles via PE transpose (fp32)
                winT_sb = pa.tile([hid, n_kt, 128], f32, tag="winT")
                for t in range(n_kt):
                    ps = pap.tile([hid, 128], f32, tag="ps_a")
                    nc.tensor.matmul(
                        ps, lhsT=win_sb[:, t, :], rhs=id_f, start=True, stop=True
                    )
                    nc.vector.tensor_copy(winT_sb[:, t, :], ps)
                # W2 = W_in @ W_cls  [in_ch, n_cls]
                w2_sb = pa.tile([128, n_kt, n_cls], f32, tag="w2")
                for t in range(n_kt):
                    ps = pap.tile([128, n_cls], f32, tag="ps_a")
                    nc.tensor.matmul(
                        ps, lhsT=winT_sb[:, t, :], rhs=wcls_sb, start=True, stop=True
                    )
                    nc.vector.tensor_copy(w2_sb[:, t, :], ps)
                # b2 = b_in @ W_cls, broadcast to [128, n_cls]
                ps_b2 = pap.tile([1, n_cls], f32, tag="ps_a")
                nc.tensor.matmul(ps_b2, lhsT=bin_sb, rhs=wcls_sb, start=True, stop=True)
                b2row = pa.tile([1, n_cls], f32, tag="b2row")
                nc.vector.tensor_copy(b2row, ps_b2)
                ps_b2b = pap.tile([128, n_cls], f32, tag="ps_a")
                nc.tensor.matmul(ps_b2b, lhsT=ones_sb, rhs=b2row, start=True, stop=True)
                b2b_sb = pa.tile([128, n_cls], f32, tag="b2b")
                nc.vector.tensor_copy(b2b_sb, ps_b2b)

                for it in range(n_it):
                    i0 = it * IW
                    p = min(IW, rows - i0)
                    x_t = pa.tile([128, in_ch], f32, tag="x_t")
                    nc.sync.dma_start(out=x_t[:p], in_=x_h[i0 : i0 + p, :])
                    xT = pa.tile([128, n_kt, 128], f32, tag="xT")
                    for t in range(n_kt):
                        ps = pap.tile([128, 128], f32, tag="ps_a")
                        nc.tensor.matmul(
                            ps[:, :p],
                            lhsT=x_t[:p, t * 128 : (t + 1) * 128],
                            rhs=id_f[:p, :p],
                            start=True,
                            stop=True,
                        )
                        nc.vector.tensor_copy(xT[:, t, :p], ps[:, :p])
                    ps_g = pap.tile([128, n_cls], f32, tag="ps_a")
                    for t in range(n_kt):
                        nc.tensor.matmul(
                            ps_g[:p],
                            lhsT=xT[:, t, :p],
                            rhs=w2_sb[:, t, :],
                            start=(t == 0),
                            stop=(t == n_kt - 1),
                        )
                    g_t = pa.tile([128, gc], bf16, tag="g_t")
                    nc.vector.tensor_add(g_t[:p, 0:n_cls], ps_g[:p], b2b_sb[:p])
                    nc.vector.memset(g_t[:p, n_cls:gc], 1.0)
                    nc.sync.dma_start(out=g_local_h[i0 : i0 + p, :], in_=g_t[:p])

            nc.gpsimd.collective_compute(
                "AllGather",
                mybir.AluOpType.bypass,
                replica_groups=[list(range(n_cores))],
                ins=[g_local_h[:]],
                outs=[g_full_h[:]],
            )

            full_jt = n_nodes // JW
            jtail = n_nodes - full_jt * JW
            if full_jt:
                nc.sync.dma_start(
                    out=G_sb[:, 0:full_jt, :],
                    in_=g_full_h[0 : full_jt * JW, :].rearrange(
                        "(jt p) c -> p jt c", p=JW
                    ),
                )
            if jtail:
                nc.sync.dma_start(
                    out=G_sb[0:jtail, full_jt, :],
                    in_=g_full_h[full_jt * JW : n_nodes, :],
                )

            # ---- Phase B: stream adj, transpose on PE, accumulate out.T ----
            # i-tiles are processed in groups of `group_its` (group width gw up
            # to 512). For each j-tile, the group's transposed blocks share one
            # PSUM tile (disjoint column ranges), so a single copy moves
            # [jw, gw] to SBUF and the accumulating matmul streams gw columns.
            n_groups = _ceil_div(n_it, group_its)
            sbufs = (3 * n_jt) // 2 if strip_bufs is None else strip_bufs
            group_w_max = min(group_its * IW, 512)
            assert group_its * IW <= 512
            pt_dt = bf16 if use_is_transpose else f32
            with (
                tc.tile_pool(name="nat", bufs=2 * group_its) as nat_pool,
                tc.tile_pool(name="strip", bufs=sbufs) as strip_pool,
                tc.tile_pool(name="outp", bufs=3) as out_pool,
                tc.tile_pool(name="pt", bufs=4, space="PSUM") as pt_pool,
                tc.tile_pool(name="acc", bufs=2, space="PSUM") as acc_pool,
                tc.tile_pool(name="fin", bufs=2, space="PSUM") as fin_pool,
            ):
                copy_ctr = 0
                for grp in range(n_groups):
                    its = list(range(grp * group_its, min((grp + 1) * group_its, n_it)))
                    widths = [min(IW, rows - it * IW) for it in its]
                    offs = [sum(widths[:k]) for k in range(len(its))]
                    gw = sum(widths)
                    strips = [
                        strip_pool.tile([128, group_w_max], bf16, tag="strip", name="strip")
                        for _ in range(n_jt)
                    ]
                    # chunk-major loads so each chunk's j-tiles can be
                    # transposed across the whole group as soon as it lands
                    nat_tiles = {}
                    for c in range(n_chunks):
                        j0 = c * stage_cols
                        cw = min(stage_cols, n_nodes - j0)
                        for k, it in enumerate(its):
                            i0 = it * IW
                            p = widths[k]
                            nt_ = nat_pool.tile([128, stage_cols], bf16, tag="nat")
                            nc.gpsimd.dma_start(
                                out=nt_[:p, :cw], in_=adj_h[i0 : i0 + p, j0 : j0 + cw]
                            )
                            nat_tiles[(k, c)] = nt_
                        jt_lo = c * stage_jtiles
                        jt_hi = min((c + 1) * stage_jtiles, n_jt)
                        for jt in range(jt_lo, jt_hi):
                            jw = min(JW, n_nodes - jt * JW)
                            off = jt * JW - j0
                            ps = pt_pool.tile([128, group_w_max], pt_dt, tag="pt")
                            for k in range(len(its)):
                                p = widths[k]
                                go = offs[k]
                                if use_is_transpose:
                                    nc.tensor.matmul(
                                        ps[:jw, go : go + p],
                                        lhsT=nat_tiles[(k, c)][:p, off : off + jw],
                                        rhs=id_b[:p, :p],
                                        is_transpose=True,
                                    )
                                else:
                                    nc.tensor.matmul(
                                        ps[:jw, go : go + p],
                                        lhsT=nat_tiles[(k, c)][:p, off : off + jw],
                                        rhs=id_b[:p, :p],
                                        start=True,
                                        stop=True,
                                    )
                            st = strips[jt]
                            if act_copy_every and copy_ctr % act_copy_every == (
                                act_copy_every - 1
                            ):
                                nc.scalar.copy(st[:jw, :gw], ps[:jw, :gw])
                            else:
                                nc.vector.tensor_copy(st[:jw, :gw], ps[:jw, :gw])
                            copy_ctr += 1
                    # accumulate out.T[:, group cols] over all j tiles
                    ps_acc = acc_pool.tile([gc, group_w_max], f32, tag="acc")
                    for jt in range(n_jt):
                        jw = min(JW, n_nodes - jt * JW)
                        nc.tensor.matmul(
                            ps_acc[:, :gw],
                            lhsT=G_sb[:jw, jt, :],
                            rhs=strips[jt][:jw, :gw],
                            start=(jt == 0),
                            stop=(jt == n_jt - 1),
                        )
                    # finalize per i-tile: U.T, scale by 1/(deg+1), + b_cls
                    U_sb = out_pool.tile([gc, group_w_max], f32, tag="U")
                    nc.vector.tensor_copy(U_sb[:, :gw], ps_acc[:, :gw])
                    for k, it in enumerate(its):
                        i0 = it * IW
                        p = widths[k]
                        go = offs[k]
                        ps_f = fin_pool.tile([128, gc], f32, tag="fin")
                        nc.tensor.matmul(
                            ps_f[:p, :],
                            lhsT=U_sb[:, go : go + p],
                            rhs=id_f[:gc, :gc],
                            start=True,
                            stop=True,
                        )
                        deg1 = out_pool.tile([128, 1], f32, tag="deg1")
                        nc.vector.tensor_scalar_add(deg1[:p], ps_f[:p, n_cls:gc], 1.0)
                        rcp = out_pool.tile([128, 1], f32, tag="rcp")
                        nc.vector.reciprocal(rcp[:p], deg1[:p])
                        o_sb = out_pool.tile([128, n_cls], f32, tag="o")
                        nc.vector.scalar_tensor_tensor(
                            out=o_sb[:p],
                            in0=ps_f[:p, 0:n_cls],
                            scalar=rcp[:p],
                            in1=bcls_sb[:p],
                            op0=mult,
                            op1=add,
                        )
                        nc.sync.dma_start(out=out_h[i0 : i0 + p, :], in_=o_sb[:p])

    nc.compile()
    return nc


_CACHE = {}


def _get_nc():
    if "nc" not in _CACHE:
        _CACHE["nc"] = build_gnn()
    return _CACHE["nc"]


def kernel(x, adj, W_in, b_in, W_cls, b_cls):
    x = np.asarray(x, dtype=np.float32)
    adj = np.asarray(adj, dtype=np.float32)
    W_in = np.asarray(W_in, dtype=np.float32)
    b_in = np.asarray(b_in, dtype=np.float32)
    W_cls = np.asarray(W_cls, dtype=np.float32)
    b_cls = np.asarray(b_cls, dtype=np.float32)

    nc = _get_nc()
    rows = adj.shape[0] // N_CORES
    in_maps = []
    for c in range(N_CORES):
        sl = slice(c * rows, (c + 1) * rows)
        in_maps.append(
            {
                "adj_blk": np.ascontiguousarray(adj[sl]),
                "x_blk": np.ascontiguousarray(x[sl]),
                "W_in": W_in,
                "b_in": b_in,
                "W_cls": W_cls,
                "b_cls": b_cls,
            }
        )
    res = run_bass_kernel_spmd(nc, in_maps, core_ids=list(range(N_CORES)))
    outs = [res.results[c]["out_blk"] for c in range(N_CORES)]
    return np.concatenate(outs, axis=0).astype(np.float32)
